# revision 1
# baseline (speedup 1.0000x reference)
"""Trainium2 Bass kernel for nn_CSWALayer (CSWA sparse-attention layer).

Strategy: pure data-parallel over batch (32 samples -> 8 cores x 4 samples).
All convs are PE matmuls (float32r, 1 cyc/row) over padded SBUF buffers with
strided window access patterns; SiLU+bias fused on ACT engine.  Attention uses
a quadrant-fold of f11 (the 2x2-tiled attention map means the AV matmul can
pre-sum the four f11 quadrants), PE transposes for the [l,c] operands, and an
exact softmax (sum over the tiled 400 logits = 4x the sum over 100).
"""

import os
import sys

for _p in ("/root/.axon_site/_ro/trn_rl_repo", "/opt/trn_rl_repo"):
    if os.path.isdir(_p) and _p not in sys.path:
        sys.path.append(_p)

import numpy as np

import concourse.bass as bass
import concourse.tile as tile
from concourse import bacc, mybir
from concourse.bass_utils import run_bass_kernel_spmd
import concourse.bass_utils as _bu

_orig_gwa = _bu.get_walrus_args


def _gwa_ldwopt(*a, **k):
    return ["--enable-ldw-opt=true" if x == "--enable-ldw-opt=false" else x
            for x in _orig_gwa(*a, **k)]


_bu.get_walrus_args = _gwa_ldwopt

F32 = mybir.dt.float32
F32R = mybir.dt.float32r
BF16 = mybir.dt.bfloat16

N_CORES = 8
B = 32
S = B // N_CORES  # samples per core

# conv dtype mode: "f32r" (full-rate fp32) or "f32" (4x slower, exact)
CONV_DT = BF16


def _mm_cast(ap):
    return ap


def _border_memset(nc, t3, H, W, zsrc):
    """Zero the 1-px border of a [128, H+2, W+2] padded tile via DMA
    from a zeros DRAM tensor (f32r-safe; Memset doesn't codegen for f32r)."""
    nc.sync.dma_start(t3[:, 0, :], zsrc[:, : W + 2])
    nc.sync.dma_start(t3[:, H + 1, :], zsrc[:, : W + 2])
    nc.sync.dma_start(t3[:, 1 : H + 1, 0], zsrc[:, :H])
    nc.sync.dma_start(t3[:, 1 : H + 1, W + 1], zsrc[:, :H])


def _conv3x3(nc, psum_pool, items, w_tiles, n_coc, co_total, W,
             apply_fn, tag, group=4):
    """3x3 same-pad conv, weight-major: each weight tile is loaded once per
    group of `group` items, and consecutive matmuls reuse it (walrus
    ldw-opt elides the redundant LDWEIGHTS).

    items: list of (src_tiles, (r0, nr), key) where src_tiles is the
    per-ci-chunk padded [128, H+2, W+2] list for that item.
    w_tiles: per-ci-chunk [128, 9*co_total] in (tap, co) layout.
    apply_fn(key, coc, r0, nr, ps)."""
    n_cic = len(items[0][0])
    n_acc = n_cic * 9
    for coc in range(n_coc):
        for g0 in range(0, len(items), group):
            grp = items[g0:g0 + group]
            pss = [psum_pool.tile([128, nr, W], F32, tag=tag, name="ps")
                   for (_, (r0, nr), _) in grp]
            k = 0
            for cic in range(n_cic):
                for t in range(9):
                    dy, dx = t // 3, t % 3
                    lhsT = w_tiles[cic][:, t * co_total + coc * 128:
                                        t * co_total + coc * 128 + 128]
                    for (srcs, (r0, nr), _), ps in zip(grp, pss):
                        rhs = srcs[cic][:, r0 + dy: r0 + dy + nr, dx: dx + W]
                        nc.tensor.matmul(ps[:], lhsT, rhs,
                                         start=(k == 0), stop=(k == n_acc - 1))
                    k += 1
            for (_, (r0, nr), key), ps in zip(grp, pss):
                apply_fn(key, coc, r0, nr, ps)


def build_program():
    nc = bacc.Bacc("TRN2", target_bir_lowering=False, debug=False,
                   num_devices=N_CORES)

    # ---- DRAM parameters (per core) ----
    dp = nc.declare_dram_parameter
    f1_d = dp("feature1", [S, 128, 80, 80], F32, isOutput=False)
    f2_d = dp("feature2", [S, 256, 40, 40], F32, isOutput=False)
    f3_d = dp("feature3", [S, 512, 20, 20], F32, isOutput=False)
    w1_d = dp("w1h", [128, 9 * 128], BF16, isOutput=False)
    w2a_d = dp("w2ah", [2, 128, 9 * 256], BF16, isOutput=False)
    w2b_d = dp("w2bh", [2, 128, 9 * 128], BF16, isOutput=False)
    w3a_d = dp("w3ah", [4, 128, 9 * 512], BF16, isOutput=False)
    w3b_d = dp("w3bh", [4, 128, 9 * 256], BF16, isOutput=False)
    w3c_d = dp("w3ch", [2, 128, 9 * 128], BF16, isOutput=False)
    wd1_d = dp("wd1h", [128, 512], BF16, isOutput=False)
    wd2_d = dp("wd2h", [128, 4, 2], BF16, isOutput=False)
    bias_d = dp("biash", [128, 11], F32, isOutput=False)
    id_d = dp("identh", [128, 128], F32, isOutput=False)
    idb_d = dp("identbh", [128, 128], BF16, isOutput=False)
    z_d = dp("zerosh", [128, 82], BF16, isOutput=False)
    out_d = dp("out", [S, 2, 400], F32, isOutput=True)

    SILU = mybir.ActivationFunctionType.Silu
    EXP = mybir.ActivationFunctionType.Exp
    RELU = mybir.ActivationFunctionType.Relu
    INV_SQRT_D = 1.0 / float(np.sqrt(2048.0))

    def packed_view(ap, yb, xb):
        # [128, 16*yb*xb] packed (ky,kx,yb,xb) view as 5d
        return ap.rearrange("p (ky kx yb xb) -> p ky kx yb xb",
                            ky=4, kx=4, yb=yb, xb=xb)

    def psum_patch_view(ap, yb, xb):
        # [128, 4*yb, 4*xb] psum tile as 5d (ky,kx,yb,xb) view
        return ap.rearrange("p (yb ky) (xb kx) -> p ky kx yb xb",
                            yb=yb, ky=4, xb=xb, kx=4)

    with tile.TileContext(nc) as tc:
        with tc.tile_pool(name="persist", bufs=1) as P:
            biast = P.tile([128, 11], F32, tag="bias")
            nc.sync.dma_start(biast[:], bias_d.ap()[:])

            f11q = [P.tile([128, 40, 40], F32, name="f11q", tag=f"f11q{s}")
                    for s in range(S)]
            f22p = [P.tile([128, 1600], F32, name="f22p", tag=f"f22p{s}")
                    for s in range(S)]
            f33p = [P.tile([128, 400], F32, name="f33p", tag=f"f33p{s}")
                    for s in range(S)]

            # ================= Phase A: conv1 (f1 path) =================
            with tc.tile_pool(name="phA", bufs=1) as PA, \
                 tc.tile_pool(name="psA", bufs=8, space="PSUM") as PSA:
                w1sb = PA.tile([128, 9 * 128], BF16, tag="w1")
                nc.sync.dma_start(w1sb[:], w1_d.ap()[:])
                f1pads = [PA.tile([128, 82, 82], BF16, name="f1pad",
                                  tag=f"f1pad{i}") for i in range(2)]
                for i in range(2):
                    _border_memset(nc, f1pads[i], 80, 80, z_d.ap())
                f1st = [PA.tile([128, 80, 80], F32, name="f1st",
                                tag=f"f1st{i}") for i in range(2)]
                for s in range(S):
                    fp = f1pads[s % 2]
                    st = f1st[s % 2]
                    nc.sync.dma_start(st[:], f1_d.ap()[s])
                    nc.vector.tensor_copy(fp[:, 1:81, 1:81], st[:])

                    def apply1(key, coc, r0, nr, ps, s=s):
                        tmp = PA.tile([128, nr, 80], F32, tag="c1tmp",
                                      name="c1tmp", bufs=3)
                        nc.scalar.activation(tmp[:], ps[:], SILU,
                                             bias=biast[:, 0:1])
                        q0 = r0 % 40
                        dst = f11q[s][:, q0:q0 + nr, :]
                        if r0 < 40:
                            nc.vector.tensor_copy(dst, tmp[:, :, 0:40])
                        else:
                            nc.vector.tensor_add(dst, dst, tmp[:, :, 0:40])
                        nc.vector.tensor_add(dst, dst, tmp[:, :, 40:80])

                    items1 = [([fp], (5 * i, 5), i) for i in range(16)]
                    _conv3x3(nc, PSA, items1, [w1sb], 1, 128, 80,
                             apply1, tag="ps1", group=4)

            # -- prefetch half the conv3a weights while phase B runs --
            with tc.tile_pool(name="w3aP", bufs=1) as W3A:
                w3asb01 = [W3A.tile([128, 9 * 512], BF16, name="w3a",
                                    tag=f"w3a{c}") for c in range(2)]
                for c in range(2):
                    nc.sync.dma_start(w3asb01[c][:], w3a_d.ap()[c])

                # ============= Phase B: conv2a, conv2b (f2 path) =========
                with tc.tile_pool(name="phB", bufs=1) as PB, \
                     tc.tile_pool(name="psB", bufs=8, space="PSUM") as PSB:
                    w2asb = [PB.tile([128, 9 * 256], BF16, name="w2a",
                                     tag=f"w2a{c}") for c in range(2)]
                    w2bsb = [PB.tile([128, 9 * 128], BF16, name="w2b",
                                     tag=f"w2b{c}") for c in range(2)]
                    for c in range(2):
                        nc.sync.dma_start(w2asb[c][:], w2a_d.ap()[c])
                        nc.sync.dma_start(w2bsb[c][:], w2b_d.ap()[c])
                    f2pads = [PB.tile([128, 42, 42], BF16, name="f2pad",
                                      tag=f"f2pad{c}") for c in range(2)]
                    f2apad = [PB.tile([128, 42, 42], BF16, name="f2apad",
                                      tag=f"f2apad{c}") for c in range(2)]
                    f22t = PB.tile([128, 40, 40], F32, tag="f22t")
                    for c in range(2):
                        _border_memset(nc, f2pads[c], 40, 40, z_d.ap())
                        _border_memset(nc, f2apad[c], 40, 40, z_d.ap())
                    f2st = [PB.tile([128, 40, 40], F32, name="f2st",
                                    tag=f"f2st{c}") for c in range(2)]
                    for s in range(S):
                        for c in range(2):
                            nc.sync.dma_start(
                                f2st[c][:],
                                f2_d.ap()[s, 128 * c:128 * (c + 1)])
                            nc.vector.tensor_copy(
                                f2pads[c][:, 1:41, 1:41], f2st[c][:])

                        def apply2a(key, coc, r0, nr, ps):
                            nc.scalar.activation(
                                f2apad[coc][:, 1 + r0:1 + r0 + nr, 1:41],
                                ps[:], SILU, bias=biast[:, 1 + coc:2 + coc])

                        items2a = [(f2pads, (10 * i, 10), i) for i in range(4)]
                        _conv3x3(nc, PSB, items2a, w2asb, 2, 256, 40,
                                 apply2a, tag="ps2", group=4)

                        def apply2b(key, coc, r0, nr, ps, s=s):
                            nc.scalar.activation(f22t[:, r0:r0 + nr, :],
                                                 ps[:], SILU,
                                                 bias=biast[:, 3:4])

                        items2b = [(f2apad, (8 * i, 8), i) for i in range(5)]
                        _conv3x3(nc, PSB, items2b, w2bsb, 1, 128, 40,
                                 apply2b, tag="ps2", group=5)
                        nc.vector.tensor_copy(
                            packed_view(f22p[s][:], 10, 10),
                            f22t[:].rearrange(
                                "p (yb ky) (xb kx) -> p ky kx yb xb",
                                yb=10, ky=4, xb=10, kx=4))

                # ============= Phase C: conv3a/b/c (f3 path) =============
                # All conv3 weights fit in SBUF at once in bf16: load w3a23 +
                # w3b + w3c upfront, so there are no inter-layer load stalls.
                with tc.tile_pool(name="phC", bufs=1) as PC:
                    f3pads = [PC.tile([128, 22, 22], BF16, name="f3pad",
                                      tag=f"f3pad{c}") for c in range(4)]
                    f3apad = [[PC.tile([128, 22, 22], BF16, name="f3apad",
                                       tag=f"f3apad{s}_{c}") for c in range(4)]
                              for s in range(S)]
                    f3bpad = [[PC.tile([128, 22, 22], BF16, name="f3bpad",
                                       tag=f"f3bpad{s}_{c}") for c in range(2)]
                              for s in range(S)]
                    f33t = [PC.tile([128, 20, 20], F32, name="f33t",
                                    tag=f"f33t{s}") for s in range(S)]
                    f3st = [PC.tile([128, 20, 20], F32, name="f3st",
                                    tag=f"f3st{c}") for c in range(4)]
                    for c in range(4):
                        _border_memset(nc, f3pads[c], 20, 20, z_d.ap())
                    for s in range(S):
                        for c in range(4):
                            _border_memset(nc, f3apad[s][c], 20, 20, z_d.ap())
                        for c in range(2):
                            _border_memset(nc, f3bpad[s][c], 20, 20, z_d.ap())

                    # phase-D pools open early: no SBUF WAR stall at D start
                    with tc.tile_pool(name="phD", bufs=1) as PD, \
                         tc.tile_pool(name="phD2", bufs=3) as PD2:
                        wd1sb = PD.tile([128, 512], BF16, tag="wd1")
                        nc.sync.dma_start(wd1sb[:], wd1_d.ap()[:])
                        wd2sb = PD.tile([128, 4, 2], BF16, tag="wd2")
                        nc.sync.dma_start(wd2sb[:], wd2_d.ap()[:])
                        ident = PD.tile([128, 128], F32, tag="ident")
                        nc.sync.dma_start(ident[:], id_d.ap()[:])
                        identb = PD.tile([128, 128], BF16, tag="identb")
                        nc.sync.dma_start(identb[:], idb_d.ap()[:])

                        with tc.tile_pool(name="w3rest", bufs=1) as W3R, \
                             tc.tile_pool(name="psC", bufs=8,
                                          space="PSUM") as PSC:
                            w3asb23 = [W3R.tile([128, 9 * 512], BF16,
                                                name="w3a2", tag=f"w3a2{c}")
                                       for c in range(2)]
                            w3bsb = [W3R.tile([128, 9 * 256], BF16,
                                              name="w3b", tag=f"w3b{c}")
                                     for c in range(4)]
                            w3csb = [W3R.tile([128, 9 * 128], BF16,
                                              name="w3c", tag=f"w3c{c}")
                                     for c in range(2)]
                            for c in range(2):
                                nc.sync.dma_start(w3asb23[c][:],
                                                  w3a_d.ap()[2 + c])
                            for c in range(4):
                                nc.sync.dma_start(w3bsb[c][:], w3b_d.ap()[c])
                            for c in range(2):
                                nc.sync.dma_start(w3csb[c][:], w3c_d.ap()[c])
                            w3asb = w3asb01 + w3asb23

                            def apply3a(key, coc, r0, nr, ps):
                                nc.scalar.activation(
                                    f3apad[key][coc][:, 1:21, 1:21], ps[:],
                                    SILU, bias=biast[:, 4 + coc:5 + coc])

                            for s in range(S):
                                for c in range(4):
                                    nc.sync.dma_start(
                                        f3st[c][:],
                                        f3_d.ap()[s, 128 * c:128 * (c + 1)])
                                    nc.vector.tensor_copy(
                                        f3pads[c][:, 1:21, 1:21], f3st[c][:])
                                _conv3x3(nc, PSC, [(f3pads, (0, 20), s)],
                                         w3asb, 4, 512, 20, apply3a,
                                         tag="ps3", group=1)

                            def apply3b(key, coc, r0, nr, ps):
                                nc.scalar.activation(
                                    f3bpad[key][coc][:, 1:21, 1:21], ps[:],
                                    SILU, bias=biast[:, 8 + coc:9 + coc])

                            items3b = [(f3apad[s], (0, 20), s)
                                       for s in range(S)]
                            _conv3x3(nc, PSC, items3b, w3bsb, 2, 256, 20,
                                     apply3b, tag="ps3", group=4)

                            def apply3c(key, coc, r0, nr, ps):
                                nc.scalar.activation(f33t[key][:], ps[:],
                                                     SILU,
                                                     bias=biast[:, 10:11])

                            items3c = [(f3bpad[s], (0, 20), s)
                                       for s in range(S)]
                            _conv3x3(nc, PSC, items3c, w3csb, 1, 128, 20,
                                     apply3c, tag="ps3", group=4)
                            for s in range(S):
                                nc.vector.tensor_copy(
                                    packed_view(f33p[s][:], 5, 5),
                                    f33t[s][:].rearrange(
                                        "p (yb ky) (xb kx) -> p ky kx yb xb",
                                        yb=5, ky=4, xb=5, kx=4))

                        # ===== Phase D: attention + head =====
                        with tc.tile_pool(name="psD", bufs=1,
                                          space="PSUM") as PSD:
                            for s in range(S):
                                # ---- QK: att[25,100] = sum_r f33p_r.T @ f22p_r ----
                                attps = PSD.tile([25, 100], F32, tag="smallps", bufs=4,
                                                 name="attps")
                                for r in range(16):
                                    nc.tensor.matmul(
                                        attps[:], f33p[s][:, 25 * r:25 * (r + 1)],
                                        f22p[s][:, 100 * r:100 * (r + 1)],
                                        start=(r == 0), stop=(r == 15))
                                # ---- softmax over tiled 400 == exp/(4*sum_100) ----
                                negmax = PD2.tile([25, 1], F32, tag="negmax")
                                nc.vector.tensor_reduce(negmax[:], attps[:],
                                                        axis=mybir.AxisListType.X,
                                                        op=mybir.AluOpType.max,
                                                        negate=True)
                                nmsc = PD2.tile([25, 1], F32, tag="nmsc")
                                nc.vector.tensor_scalar_mul(nmsc[:], negmax[:], INV_SQRT_D)
                                sm = PD2.tile([25, 100], F32, tag="sm")
                                sume = PD2.tile([25, 1], F32, tag="sume")
                                nc.scalar.activation(sm[:], attps[:], EXP, bias=nmsc[:],
                                                     scale=INV_SQRT_D, accum_out=sume[:])
                                rec = PD2.tile([25, 1], F32, tag="rec")
                                nc.vector.tensor_scalar_mul(rec[:], sume[:], 4.0)
                                nc.vector.reciprocal(rec[:], rec[:])
                                nc.vector.tensor_scalar_mul(sm[:], sm[:], rec[:])
                                # ---- smT [100, 25] ----
                                smtps = PSD.tile([100, 25], F32, tag="smallps", bufs=4,
                                                 name="smtps")
                                nc.tensor.transpose(smtps[:], sm[:], ident[:25, :25])
                                smt = PD2.tile([100, 25], BF16, tag="smt")
                                nc.vector.tensor_copy(smt[:], smtps[:])
                                # ---- pack f11q -> [c, (ky,kx,y,x)] ----
                                f11qp = PD2.tile([128, 1600], BF16, tag="f11qp")
                                nc.vector.tensor_copy(
                                    f11qp[:].rearrange(
                                        "p (ky kx y x) -> p ky kx y x",
                                        ky=4, kx=4, y=10, x=10),
                                    f11q[s][:].rearrange(
                                        "p (y ky) (x kx) -> p ky kx y x",
                                        y=10, ky=4, x=10, kx=4))
                                # ---- AV: per r, t_r[c, q] ----
                                tfin = PD2.tile([128, 400], BF16, tag="tfin")
                                for r in range(16):
                                    trps = PSD.tile([100, 128], BF16, tag="trps", bufs=2,
                                                    name="trps")
                                    nc.tensor.transpose(
                                        trps[:], f11qp[:, 100 * r:100 * (r + 1)],
                                        identb[:])
                                    trsb = PD2.tile([100, 128], BF16, tag="trsb")
                                    nc.vector.tensor_copy(trsb[:], trps[:])
                                    avps = PSD.tile([128, 25], F32, tag="smallps", bufs=4,
                                                    name="avps")
                                    nc.tensor.matmul(avps[:], trsb[:], smt[:],
                                                     start=True, stop=True)
                                    nc.scalar.copy(tfin[:, 25 * r:25 * (r + 1)], avps[:])
                                # ---- head: out = wd2 @ relu(wd1 @ t) ----
                                hk = []
                                for c in range(4):
                                    hps = PSD.tile([128, 400], F32, tag="hps", bufs=2,
                                                   name="hps")
                                    nc.tensor.matmul(hps[:],
                                                     wd1sb[:, 128 * c:128 * (c + 1)],
                                                     tfin[:], start=True, stop=True)
                                    hsb = PD2.tile([128, 400], BF16, tag=f"hsb{c}",
                                                   name="hsb")
                                    nc.scalar.activation(hsb[:], hps[:], RELU)
                                    hk.append(hsb)
                                ops = PSD.tile([2, 400], F32, tag="smallps", bufs=4,
                                               name="ops")
                                for c in range(4):
                                    nc.tensor.matmul(ops[:], wd2sb[:, c, :],
                                                     hk[c][:], start=(c == 0),
                                                     stop=(c == 3))
                                osb = PD2.tile([2, 400], F32, tag="osb")
                                nc.scalar.copy(osb[:], ops[:])
                                nc.sync.dma_start(out_d.ap()[s], osb[:])

    nc.finalize()
    return nc


def prep_weights(inputs):
    """Host-side: fold BN scale into weights, transpose to lhsT layouts."""
    import ml_dtypes
    BF = ml_dtypes.bfloat16

    def t4(w, s):
        # [co, ci, kh, kw] * s[co] -> [n_cic, 128, 9*co] in (ci | tap, co)
        w = (w * s[:, None, None, None]).astype(np.float32)
        co, ci = w.shape[0], w.shape[1]
        h = w.transpose(1, 2, 3, 0).reshape(ci, 9 * co)  # [ci, (ky,kx,co)]
        return np.ascontiguousarray(h.reshape(ci // 128, 128, 9 * co)).astype(BF)

    i = inputs
    one = lambda n: np.ones(n, np.float32)
    m = {}
    m["w1h"] = t4(i["w1"], i.get("s1", one(128)))[0]
    m["w2ah"] = t4(i["w2a"], i.get("s2a", one(256)))
    m["w2bh"] = t4(i["w2b"], i.get("s2b", one(128)))
    m["w3ah"] = t4(i["w3a"], i.get("s3a", one(512)))
    m["w3bh"] = t4(i["w3b"], i.get("s3b", one(256)))
    m["w3ch"] = t4(i["w3c"], i.get("s3c", one(128)))
    m["wd1h"] = np.ascontiguousarray(
        i["wd1"].reshape(512, 128).T.astype(np.float32)).astype(BF)  # [ci, co]
    m["wd2h"] = np.ascontiguousarray(
        i["wd2"].reshape(2, 512).T.reshape(4, 128, 2)
        .transpose(1, 0, 2).astype(np.float32)).astype(BF)        # [128,4,2]
    bias = np.zeros((128, 11), np.float32)
    cols = [("b1", 128, 0)]
    bias[:, 0] = i["b1"]
    bias[:, 1] = i["b2a"][:128]
    bias[:, 2] = i["b2a"][128:]
    bias[:, 3] = i["b2b"]
    for c in range(4):
        bias[:, 4 + c] = i["b3a"][128 * c:128 * (c + 1)]
    bias[:, 8] = i["b3b"][:128]
    bias[:, 9] = i["b3b"][128:]
    bias[:, 10] = i["b3c"]
    m["biash"] = bias
    m["identh"] = np.eye(128, dtype=np.float32)
    m["identbh"] = np.eye(128, dtype=np.float32).astype(BF)
    m["zerosh"] = np.zeros((128, 82), np.float32).astype(BF)
    return m


_NC_CACHE = None


def kernel(**inputs):
    global _NC_CACHE
    if _NC_CACHE is None:
        _NC_CACHE = build_program()
    nc = _NC_CACHE

    wmap = prep_weights(inputs)
    f1 = np.asarray(inputs["feature1"], np.float32)
    f2 = np.asarray(inputs["feature2"], np.float32)
    f3 = np.asarray(inputs["feature3"], np.float32)

    in_maps = []
    for c in range(N_CORES):
        sl = slice(S * c, S * (c + 1))
        im = dict(wmap)
        im["feature1"] = np.ascontiguousarray(f1[sl])
        im["feature2"] = np.ascontiguousarray(f2[sl])
        im["feature3"] = np.ascontiguousarray(f3[sl])
        in_maps.append(im)

    res = run_bass_kernel_spmd(nc, in_maps, list(range(N_CORES)))
    outs = [res.results[c]["out"].reshape(S, 2, 20, 20)
            for c in range(N_CORES)]
    out = np.concatenate(outs, axis=0)
    kernel.last_results = res
    return out



# revision 11
# speedup vs baseline: 2.2290x; 2.2290x over previous
"""Trainium2 Bass kernel for nn_CSWALayer (CSWA sparse-attention layer).

Strategy: pure data-parallel over batch (32 samples -> 8 cores x 4 samples).
Convs run as fp8-e4m3 DoubleRow PE matmuls (0.5 cyc/row = 2x bf16): each
matmul contracts two (ci-chunk, tap) k-slices at once, expressed as
[128, 2, N] flat-window access patterns over padded fp8 SBUF tiles (the two
pad columns per image row compute garbage that is skipped when applying
SiLU).  Per-tensor pow2 weight scales are folded out through the SiLU
activation's scale operand.  All pad / staging tiles are persistent and
zeroed once at startup; weights and zeros load on the gpsimd DMA queue while
inputs stream on the sync queue.  The per-sample attention (quadrant-folded
AV, exact softmax over the 2x2-tiled logits) is software-pipelined one
sample behind the convs so its scalar / vector latency hides under the next
sample's conv matmuls.
"""

import os
import sys

for _p in ("/root/.axon_site/_ro/trn_rl_repo", "/opt/trn_rl_repo"):
    if os.path.isdir(_p) and _p not in sys.path:
        sys.path.append(_p)

import numpy as np

import concourse.bass as bass
import concourse.tile as tile
from concourse import bacc, mybir
from concourse.bass_utils import run_bass_kernel_spmd

F32 = mybir.dt.float32
BF16 = mybir.dt.bfloat16
E4 = mybir.dt.float8e4
DR = mybir.MatmulPerfMode.DoubleRow

N_CORES = 8
B = 32
S = B // N_CORES  # samples per core

SILU = mybir.ActivationFunctionType.Silu
EXP = mybir.ActivationFunctionType.Exp
RELU = mybir.ActivationFunctionType.Relu
INV_SQRT_D = 1.0 / float(np.sqrt(2048.0))

DEBUG = False  # adds intermediate-tensor dumps (debugging only)

PH1 = 82 * 82   # conv1 pad plane
PH2 = 42 * 42   # conv2 pad plane
PH3 = 22 * 22   # conv3 pad plane


def _pair_rhs(padflat, off, n, stride):
    """[128, 2, n] view of a flat pad tile: k-tile 0 at off, k-tile 1 at
    off+stride."""
    ap = padflat[:, off: off + n].unsqueeze(1)
    v = ap.ap
    v[1] = [stride, 2]
    ap.ap = v
    return ap


def build_program():
    nc = bacc.Bacc("TRN2", target_bir_lowering=False, debug=False,
                   num_devices=N_CORES)

    dp = nc.declare_dram_parameter
    f1_d = dp("feature1", [S, 128, 80, 80], F32, isOutput=False)
    f2_d = dp("feature2", [S, 256, 40, 40], F32, isOutput=False)
    f3_d = dp("feature3", [S, 512, 20, 20], F32, isOutput=False)
    # fp8 weights, layout [128ci, n_cic, 9, co_total]
    w1_d = dp("w1h", [128, 9, 128], E4, isOutput=False)
    w2a_d = dp("w2ah", [128, 2, 9, 256], E4, isOutput=False)
    w2b_d = dp("w2bh", [128, 2, 9, 128], E4, isOutput=False)
    w3a_d = dp("w3ah", [128, 4, 9, 512], E4, isOutput=False)
    w3b_d = dp("w3bh", [128, 4, 9, 256], E4, isOutput=False)
    w3c_d = dp("w3ch", [128, 2, 9, 128], E4, isOutput=False)
    wd1_d = dp("wd1h", [128, 512], BF16, isOutput=False)
    wd2_d = dp("wd2h", [128, 4, 2], BF16, isOutput=False)
    bias_d = dp("biash", [128, 11], F32, isOutput=False)
    scale_d = dp("scaleh", [128, 6], F32, isOutput=False)  # 1/sc per layer
    id_d = dp("identh", [128, 128], F32, isOutput=False)
    idb_d = dp("identbh", [128, 128], BF16, isOutput=False)
    z_d = dp("zerosh", [128, PH1 + 8], E4, isOutput=False)
    out_d = dp("out", [S, 2, 400], F32, isOutput=True)
    if DEBUG:
        dbg = {
            "f11qd": dp("f11qd", [S, 128, 1600], F32, isOutput=True),
            "f22pd": dp("f22pd", [S, 128, 1600], F32, isOutput=True),
            "f33pd": dp("f33pd", [S, 128, 400], F32, isOutput=True),
            "smd": dp("smd", [S, 25, 100], F32, isOutput=True),
            "tfind": dp("tfind", [S, 128, 400], F32, isOutput=True),
        }

    with tile.TileContext(nc) as tc:
        with tc.tile_pool(name="P", bufs=1) as P, \
             tc.tile_pool(name="PS", bufs=1, space="PSUM") as PS:
            # ---- persistent tiles ----
            biast = P.tile([128, 11], F32, tag="bias")
            scalet = P.tile([128, 6], F32, tag="scale")
            w1sb = P.tile([128, 9, 128], E4, tag="w1")
            w2asb = P.tile([128, 2, 9, 256], E4, tag="w2a")
            w2bsb = P.tile([128, 2, 9, 128], E4, tag="w2b")
            w3asb = P.tile([128, 4, 9, 512], E4, tag="w3a")
            w3bsb = P.tile([128, 4, 9, 256], E4, tag="w3b")
            w3csb = P.tile([128, 2, 9, 128], E4, tag="w3c")
            wd1sb = P.tile([128, 512], BF16, tag="wd1")
            wd2sb = P.tile([128, 4, 2], BF16, tag="wd2")
            ident = P.tile([128, 128], F32, tag="ident")
            identb = P.tile([128, 128], BF16, tag="identb")

            f1pad = [P.tile([128, PH1 + 8], E4, name="f1pad",
                            tag=f"f1pad{i}") for i in range(2)]
            f1st = [P.tile([128, 20, 80], F32, name="f1st", tag=f"f1st{i}")
                    for i in range(4)]
            f2st = [P.tile([128, 40, 40], F32, name="f2st", tag=f"f2st{i}")
                    for i in range(2)]
            f2pad = [P.tile([128, 2 * PH2 + 8], E4, name="f2pad",
                            tag=f"f2pad{i}") for i in range(2)]
            f2apad = [P.tile([128, 2 * PH2 + 8], E4, name="f2apad",
                             tag=f"f2apad{i}") for i in range(2)]
            f3st = [P.tile([128, 4, 20, 20], F32, name="f3st",
                           tag=f"f3st{i}") for i in range(2)]
            f3pad = [P.tile([128, 4 * PH3 + 8], E4, name="f3pad",
                            tag=f"f3pad{i}") for i in range(2)]
            f3apad = [P.tile([128, 4 * PH3 + 8], E4, name="f3apad",
                             tag=f"f3apad{i}") for i in range(2)]
            f3bpad = [P.tile([128, 2 * PH3 + 8], E4, name="f3bpad",
                             tag=f"f3bpad{i}") for i in range(2)]
            f22t = [P.tile([128, 40, 40], BF16, name="f22t",
                           tag=f"f22t{i}") for i in range(2)]
            f33t = [P.tile([128, 20, 20], BF16, name="f33t",
                           tag=f"f33t{i}") for i in range(2)]
            f22p = [P.tile([128, 16, 100], BF16, name="f22p",
                           tag=f"f22p{i}") for i in range(2)]
            f33p = [P.tile([128, 16, 25], BF16, name="f33p",
                           tag=f"f33p{i}") for i in range(2)]
            f11q = [P.tile([128, 40, 40], F32, name="f11q",
                           tag=f"f11q{i}") for i in range(2)]
            f11qp = [P.tile([128, 16, 100], BF16, name="f11qp",
                            tag=f"f11qp{i}") for i in range(2)]

            def plane(padflat, c, H, W):
                # [128, H+2, W+2] view of plane c of a flat pad tile
                ph = (H + 2) * (W + 2)
                return padflat[:, c * ph: (c + 1) * ph].rearrange(
                    "p (h w) -> p h w", h=H + 2, w=W + 2)

            # ---- startup DMAs: inputs (sync q) / weights+zeros (gpsimd q) --
            nc.gpsimd.dma_start(f1pad[0][:], z_d.ap()[:, : PH1 + 8])
            nc.gpsimd.dma_start(biast[:], bias_d.ap()[:])
            nc.gpsimd.dma_start(scalet[:], scale_d.ap()[:])
            nc.gpsimd.dma_start(w1sb[:], w1_d.ap()[:])

            def load_f1(s, k):
                nc.sync.dma_start(f1st[k][:],
                                  f1_d.ap()[s, :, 20 * k: 20 * k + 20, :])

            def load_f2(s):
                for c in range(2):
                    nc.sync.dma_start(f2st[c][:],
                                      f2_d.ap()[s, 128 * c: 128 * (c + 1)])

            def load_f3(s):
                for c in range(4):
                    nc.sync.dma_start(
                        f3st[s % 2][:, c],
                        f3_d.ap()[s, 128 * c: 128 * (c + 1)])

            def cast_f1(s, k):
                nc.vector.tensor_copy(
                    plane(f1pad[s % 2], 0, 80, 80)[
                        :, 1 + 20 * k: 21 + 20 * k, 1: 81],
                    f1st[k][:])

            def cast_f2(s):
                for c in range(2):
                    nc.scalar.copy(
                        plane(f2pad[s % 2], c, 40, 40)[:, 1: 41, 1: 41],
                        f2st[c][:])

            def cast_f3(s):
                for c in range(4):
                    nc.scalar.copy(
                        plane(f3pad[s % 2], c, 20, 20)[:, 1: 21, 1: 21],
                        f3st[s % 2][:, c])

            # sample 0 input: interleave f1 slab loads and casts
            for k in range(4):
                load_f1(0, k)
                cast_f1(0, k)

            nc.gpsimd.dma_start(f1pad[1][:], z_d.ap()[:, : PH1 + 8])
            for i in range(2):
                nc.gpsimd.dma_start(f2pad[i][:], z_d.ap()[:, : 2 * PH2 + 8])
                nc.gpsimd.dma_start(f2apad[i][:], z_d.ap()[:, : 2 * PH2 + 8])
            nc.gpsimd.dma_start(w2asb[:], w2a_d.ap()[:])
            nc.gpsimd.dma_start(w2bsb[:], w2b_d.ap()[:])
            for i in range(2):
                nc.gpsimd.dma_start(f3pad[i][:], z_d.ap()[:, : 4 * PH3 + 8])
                nc.gpsimd.dma_start(f3apad[i][:], z_d.ap()[:, : 4 * PH3 + 8])
                nc.gpsimd.dma_start(f3bpad[i][:], z_d.ap()[:, : 2 * PH3 + 8])
            nc.gpsimd.dma_start(w3asb[:], w3a_d.ap()[:])
            nc.gpsimd.dma_start(w3bsb[:], w3b_d.ap()[:])
            nc.gpsimd.dma_start(w3csb[:], w3c_d.ap()[:])
            nc.gpsimd.dma_start(wd1sb[:], wd1_d.ap()[:])
            nc.gpsimd.dma_start(wd2sb[:], wd2_d.ap()[:])
            nc.gpsimd.dma_start(ident[:], id_d.ap()[:])
            nc.gpsimd.dma_start(identb[:], idb_d.ap()[:])

            load_f2(0)
            cast_f2(0)
            load_f3(0)
            cast_f3(0)

            # ---------------- conv emission helpers ----------------
            def conv_paired(wsb, padflat, n_cic, co_total, items, W,
                            apply_fn, scale_col):
                """cic-paired DoubleRow conv over a flat pad tile.
                items: list of (r0, nr), at most 4 in flight."""
                Wp = W + 2
                phl = {80: PH1, 40: PH2, 20: PH3}[W]
                n_coc = co_total // 128
                npair = n_cic // 2
                for coc in range(n_coc):
                    pss = [PS.tile([128, nr * Wp], F32, tag="cps",
                                   name="cps", bufs=4)
                           for (r0, nr) in items]
                    k = 0
                    nacc = npair * 9
                    for j in range(npair):
                        for t in range(9):
                            dy, dx = t // 3, t % 3
                            lhsT = wsb[:, 2 * j: 2 * j + 2, t,
                                       coc * 128: coc * 128 + 128]
                            for (r0, nr), ps in zip(items, pss):
                                off = (2 * j) * phl + (r0 + dy) * Wp + dx
                                rhs = _pair_rhs(padflat, off, nr * Wp, phl)
                                nc.tensor.matmul(ps[:], lhsT, rhs,
                                                 start=(k == 0),
                                                 stop=(k == nacc - 1),
                                                 perf_mode=DR)
                            k += 1
                    for (r0, nr), ps in zip(items, pss):
                        psv = ps[:].rearrange("p (r w) -> p r w",
                                              r=nr, w=Wp)[:, :, 0: W]
                        apply_fn(coc, r0, nr, psv, scale_col)

            # conv1: tap-paired DoubleRow over single ci chunk
            TAP_PLAN = [(0, 1), (2, 3), (4, 5), (6, 7), (8,)]

            def conv1(s):
                pad = f1pad[s % 2]
                for g0 in range(0, 16, 4):
                    items = [(5 * i, 5) for i in range(g0, g0 + 4)]
                    pss = [PS.tile([128, 5 * 82], F32, tag="cps",
                                   name="c1ps", bufs=4) for _ in items]
                    for ti, taps in enumerate(TAP_PLAN):
                        t0 = taps[0]
                        dy0, dx0 = t0 // 3, t0 % 3
                        if len(taps) == 2:
                            t1 = taps[1]
                            dy1, dx1 = t1 // 3, t1 % 3
                            delta = (dy1 - dy0) * 82 + (dx1 - dx0)
                            lhsT = w1sb[:, t0: t0 + 2, :]
                            for (r0, nr), ps in zip(items, pss):
                                off = (r0 + dy0) * 82 + dx0
                                rhs = _pair_rhs(pad, off, nr * 82, delta)
                                nc.tensor.matmul(ps[:], lhsT, rhs,
                                                 start=(ti == 0), stop=False,
                                                 perf_mode=DR)
                        else:
                            lhsT = w1sb[:, t0, :]
                            for (r0, nr), ps in zip(items, pss):
                                off = (r0 + dy0) * 82 + dx0
                                rhs = pad[:, off: off + nr * 82]
                                nc.tensor.matmul(ps[:], lhsT, rhs,
                                                 start=False, stop=True)
                    for (r0, nr), ps in zip(items, pss):
                        psv = ps[:].rearrange("p (r w) -> p r w",
                                              r=nr, w=82)[:, :, 0: 80]
                        tmp = P.tile([128, 5, 80], F32, tag="c1tmp",
                                     name="c1tmp", bufs=4)
                        nc.scalar.activation(tmp[:], psv, SILU,
                                             bias=biast[:, 0:1],
                                             scale=scalet[:, 0:1])
                        q0 = r0 % 40
                        dst = f11q[s % 2][:, q0: q0 + nr, :]
                        if r0 < 40:
                            nc.vector.tensor_copy(dst, tmp[:, :, 0:40])
                        else:
                            nc.vector.tensor_add(dst, dst, tmp[:, :, 0:40])
                        nc.vector.tensor_add(dst, dst, tmp[:, :, 40:80])

            def conv2a(s):
                def apply(coc, r0, nr, psv, sc):
                    nc.scalar.activation(
                        plane(f2apad[s % 2], coc, 40, 40)[
                            :, 1 + r0: 1 + r0 + nr, 1: 41],
                        psv, SILU, bias=biast[:, 1 + coc: 2 + coc], scale=sc)
                conv_paired(w2asb, f2pad[s % 2], 2, 256,
                            [(10 * i, 10) for i in range(4)], 40,
                            apply, scalet[:, 1:2])

            def conv2b(s):
                def apply(coc, r0, nr, psv, sc):
                    nc.scalar.activation(
                        f22t[s % 2][:, r0: r0 + nr, :],
                        psv, SILU, bias=biast[:, 3:4], scale=sc)
                conv_paired(w2bsb, f2apad[s % 2], 2, 128,
                            [(10 * i, 10) for i in range(4)], 40,
                            apply, scalet[:, 2:3])
                # pack [p,(yb ky),(xb kx)] -> [p,(ky kx),(yb xb)]
                nc.vector.tensor_copy(
                    f22p[s % 2][:].rearrange(
                        "p (ky kx) (yb xb) -> p ky kx yb xb",
                        ky=4, kx=4, yb=10, xb=10),
                    f22t[s % 2][:].rearrange(
                        "p (yb ky) (xb kx) -> p ky kx yb xb",
                        yb=10, ky=4, xb=10, kx=4))

            def conv3a(s):
                def apply(coc, r0, nr, psv, sc):
                    nc.scalar.activation(
                        plane(f3apad[s % 2], coc, 20, 20)[:, 1: 21, 1: 21],
                        psv, SILU, bias=biast[:, 4 + coc: 5 + coc], scale=sc)
                conv_paired(w3asb, f3pad[s % 2], 4, 512,
                            [(0, 20)], 20, apply, scalet[:, 3:4])

            def conv3b(s):
                def apply(coc, r0, nr, psv, sc):
                    nc.scalar.activation(
                        plane(f3bpad[s % 2], coc, 20, 20)[:, 1: 21, 1: 21],
                        psv, SILU, bias=biast[:, 8 + coc: 9 + coc], scale=sc)
                conv_paired(w3bsb, f3apad[s % 2], 4, 256,
                            [(0, 20)], 20, apply, scalet[:, 4:5])

            def conv3c(s):
                def apply(coc, r0, nr, psv, sc):
                    nc.scalar.activation(f33t[s % 2][:], psv, SILU,
                                         bias=biast[:, 10:11], scale=sc)
                conv_paired(w3csb, f3bpad[s % 2], 2, 128,
                            [(0, 20)], 20, apply, scalet[:, 5:6])
                nc.vector.tensor_copy(
                    f33p[s % 2][:].rearrange(
                        "p (ky kx) (yb xb) -> p ky kx yb xb",
                        ky=4, kx=4, yb=5, xb=5),
                    f33t[s % 2][:].rearrange(
                        "p (yb ky) (xb kx) -> p ky kx yb xb",
                        yb=5, ky=4, xb=5, kx=4))

            # ---------------- attention (sample s), staged ----------------
            def att_qk(s):
                attps = PS.tile([25, 100], F32, tag="aps", bufs=4,
                                name="attps")
                for r in range(16):
                    nc.tensor.matmul(attps[:], f33p[s % 2][:, r, :],
                                     f22p[s % 2][:, r, :],
                                     start=(r == 0), stop=(r == 15))
                negmax = P.tile([25, 1], F32, tag="negmax")
                nc.vector.tensor_reduce(negmax[:], attps[:],
                                        axis=mybir.AxisListType.X,
                                        op=mybir.AluOpType.max, negate=True)
                nmsc = P.tile([25, 1], F32, tag="nmsc")
                nc.vector.tensor_scalar_mul(nmsc[:], negmax[:], INV_SQRT_D)
                sm = P.tile([25, 100], F32, tag="sm")
                sume = P.tile([25, 1], F32, tag="sume")
                nc.scalar.activation(sm[:], attps[:], EXP, bias=nmsc[:],
                                     scale=INV_SQRT_D, accum_out=sume[:])
                rec = P.tile([25, 1], F32, tag="rec")
                nc.vector.tensor_scalar_mul(rec[:], sume[:], 4.0)
                nc.vector.reciprocal(rec[:], rec[:])
                nc.vector.tensor_scalar_mul(sm[:], sm[:], rec[:])
                # pack f11q -> [c, (ky kx), (y x)] bf16
                nc.vector.tensor_copy(
                    f11qp[s % 2][:].rearrange(
                        "p (ky kx) (y x) -> p ky kx y x",
                        ky=4, kx=4, y=10, x=10),
                    f11q[s % 2][:].rearrange(
                        "p (y ky) (x kx) -> p ky kx y x",
                        y=10, ky=4, x=10, kx=4))
                return sm

            def att_av(s, sm, tfin):
                smtps = PS.tile([100, 25], F32, tag="aps", bufs=4,
                                name="smtps")
                nc.tensor.transpose(smtps[:], sm[:], ident[:25, :25])
                smt = P.tile([100, 25], BF16, tag="smt")
                nc.vector.tensor_copy(smt[:], smtps[:])
                for r in range(16):
                    trps = PS.tile([100, 128], BF16, tag="aps", bufs=4,
                                   name="trps")
                    nc.tensor.transpose(trps[:], f11qp[s % 2][:, r, :],
                                        identb[:])
                    trsb = P.tile([100, 128], BF16, tag="trsb", bufs=2,
                                  name="trsb")
                    if r % 2 == 0:
                        nc.vector.tensor_copy(trsb[:], trps[:])
                    else:
                        nc.scalar.copy(trsb[:], trps[:])
                    avps = PS.tile([128, 25], F32, tag="aps", bufs=4,
                                   name="avps")
                    nc.tensor.matmul(avps[:], trsb[:], smt[:],
                                     start=True, stop=True)
                    nc.scalar.copy(tfin[:, 25 * r: 25 * (r + 1)], avps[:])

            def att_head(s, tfin):
                hk = []
                for c in range(4):
                    hps = PS.tile([128, 400], F32, tag="cps", bufs=4,
                                  name="hps")
                    nc.tensor.matmul(hps[:], wd1sb[:, 128 * c: 128 * (c + 1)],
                                     tfin[:], start=True, stop=True)
                    hsb = P.tile([128, 400], BF16, tag=f"hsb{c}", name="hsb")
                    nc.scalar.activation(hsb[:], hps[:], RELU)
                    hk.append(hsb)
                ops = PS.tile([2, 400], F32, tag="aps", bufs=4, name="ops")
                for c in range(4):
                    nc.tensor.matmul(ops[:], wd2sb[:, c, :], hk[c][:],
                                     start=(c == 0), stop=(c == 3))
                osb = P.tile([2, 400], F32, tag="osb", bufs=2, name="osb")
                nc.scalar.copy(osb[:], ops[:])
                nc.scalar.dma_start(out_d.ap()[s], osb[:])

            # ---------------- main pipeline ----------------
            tfins = [P.tile([128, 400], BF16, tag=f"tfin{i}", name="tfin")
                     for i in range(2)]
            for i in range(S + 1):
                s, sp = i, i - 1  # conv sample / attention sample
                if s < S:
                    if s + 1 < S:  # prefetch next sample's inputs
                        for k in range(4):
                            load_f1(s + 1, k)
                        load_f2(s + 1)
                        load_f3(s + 1)
                    conv1(s)
                    if DEBUG:
                        dbgf = P.tile([128, 1600], F32, tag="dbgq", bufs=2)
                        nc.vector.tensor_copy(
                            dbgf[:], f11q[s % 2][:].rearrange(
                                "p a b -> p (a b)"))
                        nc.gpsimd.dma_start(dbg["f11qd"].ap()[s], dbgf[:])
                    if sp >= 0:
                        sm = att_qk(sp)
                    if s + 1 < S:
                        for k in range(4):
                            cast_f1(s + 1, k)
                    conv2a(s)
                    if s + 1 < S:
                        cast_f2(s + 1)
                    conv2b(s)
                    if DEBUG:
                        dbg2 = P.tile([128, 1600], F32, tag="dbg2", bufs=2)
                        nc.vector.tensor_copy(
                            dbg2[:], f22p[s % 2][:].rearrange(
                                "p a b -> p (a b)"))
                        nc.gpsimd.dma_start(dbg["f22pd"].ap()[s], dbg2[:])
                    if sp >= 0:
                        att_av(sp, sm, tfins[sp % 2])
                    conv3a(s)
                    if sp >= 0:
                        if DEBUG:
                            nc.gpsimd.dma_start(dbg["smd"].ap()[sp], sm[:])
                            dbgt = P.tile([128, 400], F32, tag="dbgt",
                                          bufs=2)
                            nc.vector.tensor_copy(dbgt[:],
                                                  tfins[sp % 2][:])
                            nc.gpsimd.dma_start(dbg["tfind"].ap()[sp],
                                                dbgt[:])
                        att_head(sp, tfins[sp % 2])
                    conv3b(s)
                    if s + 1 < S:
                        cast_f3(s + 1)
                    conv3c(s)
                    if DEBUG:
                        dbg3 = P.tile([128, 400], F32, tag="dbg3", bufs=2)
                        nc.vector.tensor_copy(
                            dbg3[:], f33p[s % 2][:].rearrange(
                                "p a b -> p (a b)"))
                        nc.gpsimd.dma_start(dbg["f33pd"].ap()[s], dbg3[:])
                else:
                    sm = att_qk(sp)
                    att_av(sp, sm, tfins[sp % 2])
                    if DEBUG:
                        nc.gpsimd.dma_start(dbg["smd"].ap()[sp], sm[:])
                        dbgt = P.tile([128, 400], F32, tag="dbgt", bufs=2)
                        nc.vector.tensor_copy(dbgt[:], tfins[sp % 2][:])
                        nc.gpsimd.dma_start(dbg["tfind"].ap()[sp], dbgt[:])
                    att_head(sp, tfins[sp % 2])

    nc.finalize()
    return nc


def prep_weights(inputs):
    """Host-side: fold BN scale into weights, quantize to e4m3 with
    per-tensor pow2 scales, transpose to [128ci, n_cic, 9, co] lhsT layout."""
    import ml_dtypes
    BF = ml_dtypes.bfloat16
    E4np = ml_dtypes.float8_e4m3

    scales = np.zeros(6, np.float64)

    def t4(w, s, li):
        # [co, ci, kh, kw] * s[co] -> [128, n_cic, 9, co] fp8
        w = np.asarray(w, np.float32) * np.asarray(s, np.float32)[:, None,
                                                                  None, None]
        sc = 2.0 ** np.floor(np.log2(224.0 / (np.abs(w).max() + 1e-30)))
        scales[li] = sc
        co, ci = w.shape[0], w.shape[1]
        h = (w * sc).transpose(1, 2, 3, 0).reshape(ci // 128, 128, 9, co)
        return np.ascontiguousarray(h.transpose(1, 0, 2, 3)).astype(E4np)

    i = {k: np.asarray(v) for k, v in inputs.items()}
    one = lambda n: np.ones(n, np.float32)
    m = {}
    m["w1h"] = t4(i["w1"], i.get("s1", one(128)), 0)[:, 0]
    m["w2ah"] = t4(i["w2a"], i.get("s2a", one(256)), 1)
    m["w2bh"] = t4(i["w2b"], i.get("s2b", one(128)), 2)
    m["w3ah"] = t4(i["w3a"], i.get("s3a", one(512)), 3)
    m["w3bh"] = t4(i["w3b"], i.get("s3b", one(256)), 4)
    m["w3ch"] = t4(i["w3c"], i.get("s3c", one(128)), 5)
    m["wd1h"] = np.ascontiguousarray(
        i["wd1"].reshape(512, 128).T.astype(np.float32)).astype(BF)
    m["wd2h"] = np.ascontiguousarray(
        i["wd2"].reshape(2, 512).T.reshape(4, 128, 2)
        .transpose(1, 0, 2).astype(np.float32)).astype(BF)
    bias = np.zeros((128, 11), np.float32)
    bias[:, 0] = i["b1"]
    bias[:, 1] = i["b2a"][:128]
    bias[:, 2] = i["b2a"][128:]
    bias[:, 3] = i["b2b"]
    for c in range(4):
        bias[:, 4 + c] = i["b3a"][128 * c: 128 * (c + 1)]
    bias[:, 8] = i["b3b"][:128]
    bias[:, 9] = i["b3b"][128:]
    bias[:, 10] = i["b3c"]
    m["biash"] = bias
    m["scaleh"] = np.broadcast_to(
        (1.0 / scales).astype(np.float32)[None, :], (128, 6)).copy()
    m["identh"] = np.eye(128, dtype=np.float32)
    m["identbh"] = np.eye(128, dtype=np.float32).astype(BF)
    m["zerosh"] = np.zeros((128, PH1 + 8), np.float32).astype(E4np)
    return m


_NC_CACHE = None


def kernel(**inputs):
    global _NC_CACHE
    if _NC_CACHE is None:
        _NC_CACHE = build_program()
    nc = _NC_CACHE

    wmap = prep_weights(inputs)
    f1 = np.asarray(inputs["feature1"], np.float32)
    f2 = np.asarray(inputs["feature2"], np.float32)
    f3 = np.asarray(inputs["feature3"], np.float32)

    in_maps = []
    for c in range(N_CORES):
        sl = slice(S * c, S * (c + 1))
        im = dict(wmap)
        im["feature1"] = np.ascontiguousarray(f1[sl])
        im["feature2"] = np.ascontiguousarray(f2[sl])
        im["feature3"] = np.ascontiguousarray(f3[sl])
        in_maps.append(im)

    res = run_bass_kernel_spmd(nc, in_maps, list(range(N_CORES)))
    outs = [res.results[c]["out"].reshape(S, 2, 20, 20)
            for c in range(N_CORES)]
    out = np.concatenate(outs, axis=0)
    kernel.last_results = res
    return out
